# revision 8
# baseline (speedup 1.0000x reference)
"""Fused self-attention (softmax over the QUERY axis) for Trainium2, 8 NeuronCores.

Problem (hardcoded shapes):
    query/key/value: [B=4, S=2048, D=1024] fp32, H=1024
    q = query @ Wq.T + bq ; k = key @ Wk.T + bk ; v = value @ Wv.T + bv
    scores = einsum('bqh,bkh->bqk', q, k) * 0.125
    attn = softmax(scores, axis=1)            # over the QUERY axis
    out  = einsum('bqk,bkh->bqh', attn, v)
    y    = out @ Wo.T + bo

Algebraic restructure (biases bq/bk are zero in this problem's setup_inputs;
a numpy fallback handles the general case):
    scores[q,k] = xq[q,:] @ G @ xk[k,:]^T      with G  = Wq^T @ Wk   [D,D]
    y[q,:]      = sum_k attn[q,k] * vw[k,:]    with vw = (xv @ Gv^T + bvo),
                  Gv = Wo @ Wv [D,D], bvo = Wo @ bv
G / Gv are computed once on the host (fp64); device work is 4 GEMM phases:
    P1: M2[d,k]   = sum_e GT[e,d] * xkT[e,k]          (GT = G^T)
    P2: sT[k,q]   = sum_d M2[d,k] * xqT[d,q] ; expT = exp(scale*sT),
                    denom[k] = sum_q expT
    P3: vw[k,d]   = sum_e xvT[e,k] * GvT[e,d] (+bvo) ; vw[k,:] *= 1/denom[k]
    P4: yT[d,q]   = sum_k vw[k,d] * expT[k,q]         (partial over keys)

Sharding: 8 cores = 4 batches x 2 key-halves (T=1024 keys/core). Softmax over
q is per-key, so key-sharding needs no cross-core reduction; the host sums the
two key-half partials of each batch and adds bo.

All matmul operands are bf16 (measured end-to-end max-err/absmax ~9e-3 vs the
2e-2 gate; PSUM accumulation stays fp32).  bf16 halves every input DMA, which
lets P1 stream e-outer against the HBM: subpasses of 5+3 PSUM banks accumulate
over e while the (gt_e, xk_e) pairs arrive, so the PE never starves at kernel
start (the fp32 predecessor idled ~13us waiting for an 8MB preload and ran the
first 26us at the cold 1.2GHz HAM clock).  P4 stores y per (md,qb) chunk so
the post-loop drain is one 256KB DMA instead of 1MB + copies.
"""

import numpy as np

import concourse.bacc as bacc
import concourse.bass as bass
import concourse.mybir as mybir
import concourse.tile as tile
from concourse.bass_utils import run_bass_kernel_spmd

P = 128
B = 4
S = 2048          # query sequence length
D = 1024          # embed dim (= hidden dim H)
T = 1024          # keys per core (half of the 2048-key sequence)
DO = D // P       # 8
TO = T // P       # 8
QB = 512          # query block width
NQB = S // QB     # 4
NB = 512
SCALE = 64 ** -0.5
NWARM = 42

F32 = mybir.dt.float32
BF16 = mybir.dt.bfloat16
AF = mybir.ActivationFunctionType


def _build_program():
    nc = bacc.Bacc(None, target_bir_lowering=False)

    xqT = nc.dram_tensor("xqT", [D, S], BF16, kind="ExternalInput")
    xkT = nc.dram_tensor("xkT", [D, T], BF16, kind="ExternalInput")
    xvT = nc.dram_tensor("xvT", [D, T], BF16, kind="ExternalInput")
    gT = nc.dram_tensor("gT", [D, D], BF16, kind="ExternalInput")    # (Wq^T Wk)^T
    gvT = nc.dram_tensor("gvT", [D, D], BF16, kind="ExternalInput")  # (Wo Wv)^T
    bvo = nc.dram_tensor("bvo", [D], F32, kind="ExternalInput")      # Wo @ bv
    y = nc.dram_tensor("y", [D, S], F32, kind="ExternalOutput")      # yT partial

    def dram3(t, cols, ncols, inner):
        # [P, DO, ncols] view of a [D, inner] dram tensor: partition stride =
        # one row, mid dim hops 128 rows, minor dim walks `ncols` columns.
        ap = t[:]
        return bass.AP(tensor=ap.tensor, offset=cols,
                       ap=[[inner, P], [P * inner, DO], [1, ncols]])

    with tile.TileContext(nc) as tc:
        with (
            tc.tile_pool(name="singles", bufs=1) as singles,
            tc.tile_pool(name="psum", bufs=8, space="PSUM") as psum,
            tc.tile_pool(name="big", bufs=1) as big,
            tc.tile_pool(name="yt_pool", bufs=4) as yt_pool,
        ):
            denom = singles.tile([P, TO, NQB], F32, tag="denom")
            dsum = singles.tile([P, TO], F32, tag="dsum")
            recip = singles.tile([P, TO], F32, tag="recip")
            bvo_sb = singles.tile([P, D], F32, tag="bvo")

            # HAM warmup: keep the PE busy while the first input DMAs land.
            wtile = singles.tile([P, P], BF16, tag="warm")
            nc.vector.memset(wtile, 0.0)
            wps = psum.tile([P, P], F32, tag="ps", name="warm_ps")
            for _ in range(NWARM):
                nc.tensor.matmul(wps, lhsT=wtile, rhs=wtile, start=True, stop=True)

            # ---- input tiles ----
            gt_t = [big.tile([P, D], BF16, tag=f"g{e}", name=f"gt{e}") for e in range(DO)]
            xk_t = [big.tile([P, T], BF16, tag=f"k{e}", name=f"xk{e}") for e in range(DO)]
            xq = big.tile([P, DO, S], BF16, tag="xq")
            m2 = big.tile([P, DO, T], BF16, tag="m2")
            expT = big.tile([P, TO, S], BF16, tag="expT")
            vw = big.tile([P, TO, D], BF16, tag="vw")
            xv = big.tile([P, DO, T], BF16, tag="xv")
            gv = big.tile([P, DO, D], BF16, tag="gv")

            # ---- DMA streams, ordered by first use.  P1's (gt_e, xkA_e)
            # pairs are interleaved across the two HWDGE queues so pair e
            # lands at ~(e+1)*1.1us; everything later trails behind them.
            for e in range(DO):
                eng_g = nc.sync if e % 2 == 0 else nc.scalar
                eng_k = nc.scalar if e % 2 == 0 else nc.sync
                eng_g.dma_start(out=gt_t[e], in_=gT[e * P:(e + 1) * P, :])
                eng_k.dma_start(out=xk_t[e][:, 0:NB],
                                in_=xkT[e * P:(e + 1) * P, 0:NB])
            for e in range(DO):
                eng = nc.sync if e % 2 == 0 else nc.scalar
                eng.dma_start(out=xk_t[e][:, NB:T],
                              in_=xkT[e * P:(e + 1) * P, NB:T])
            for qb in range(NQB):
                eng = nc.sync if qb % 2 == 0 else nc.scalar
                eng.dma_start(out=xq[:, :, qb * QB:(qb + 1) * QB],
                              in_=dram3(xqT, qb * QB, QB, S))
            bvo_ap = bvo[:]
            nc.sync.dma_start(
                out=bvo_sb,
                in_=bass.AP(tensor=bvo_ap.tensor, offset=bvo_ap.offset,
                            ap=[[0, P]] + list(bvo_ap.ap)),
            )
            nc.sync.dma_start(out=xv, in_=dram3(xvT, 0, T, T))
            nc.scalar.dma_start(out=gv, in_=dram3(gvT, 0, D, D))

            # ---- P1: M2[d,k] = sum_e GT[e,d] * xk[e,k].  The first xk half
            # runs e-outer in 6/2-bank subpasses so each e-step's compute
            # (6 x 216ns) outlasts the (gt_e, xkA_e) DMA pair interval; the
            # second half (data resident by then) runs md-outer with one
            # pipelined accumulation chain per bank.
            for mds in ((0, 1, 2, 3, 4, 5, 6), (7,)):
                pss = {md: psum.tile([P, NB], F32, tag="ps", name=f"p1a_{md}")
                       for md in mds}
                for e in range(DO):
                    for md in mds:
                        nc.tensor.matmul(
                            pss[md],
                            lhsT=gt_t[e][:, md * P:(md + 1) * P],
                            rhs=xk_t[e][:, 0:NB],
                            start=(e == 0),
                            stop=(e == DO - 1),
                        )
                for md in mds:
                    nc.vector.tensor_copy(out=m2[:, md, 0:NB], in_=pss[md])
            for md in range(DO):
                ps = psum.tile([P, NB], F32, tag="ps", name=f"p1b_{md}")
                for e in range(DO):
                    nc.tensor.matmul(
                        ps,
                        lhsT=gt_t[e][:, md * P:(md + 1) * P],
                        rhs=xk_t[e][:, NB:T],
                        start=(e == 0),
                        stop=(e == DO - 1),
                    )
                nc.vector.tensor_copy(out=m2[:, md, NB:T], in_=ps)

            # ---- P2: scores_T -> exp (bf16) + per-key partial row sums ----
            for qb in range(NQB):
                for kt in range(TO):
                    ps = psum.tile([P, QB], F32, tag="ps")
                    for dd in range(DO):
                        nc.tensor.matmul(
                            ps,
                            lhsT=m2[:, dd, kt * P:(kt + 1) * P],
                            rhs=xq[:, dd, qb * QB:(qb + 1) * QB],
                            start=(dd == 0),
                            stop=(dd == DO - 1),
                        )
                    nc.scalar.activation(
                        out=expT[:, kt, qb * QB:(qb + 1) * QB],
                        in_=ps,
                        func=AF.Exp,
                        scale=float(SCALE),
                        accum_out=denom[:, kt, qb:qb + 1],
                    )

            # softmax denominators (emitted now so the DVE computes them the
            # moment P2's last accumulator lands, ahead of P3's adds)
            nc.vector.reduce_sum(out=dsum, in_=denom, axis=mybir.AxisListType.X)
            nc.vector.reciprocal(out=recip, in_=dsum)

            # ---- P3: vw[k,d] = (sum_e xv[e,k] * GvT[e,d] + bvo) / denom ----
            for mk in range(TO):
                ps2 = [psum.tile([P, NB], F32, tag="ps", name=f"p3_{mk}_{i}")
                       for i in range(D // NB)]
                for e in range(DO):
                    for nb in range(D // NB):
                        nc.tensor.matmul(
                            ps2[nb],
                            lhsT=xv[:, e, mk * P:(mk + 1) * P],
                            rhs=gv[:, e, nb * NB:(nb + 1) * NB],
                            start=(e == 0),
                            stop=(e == DO - 1),
                        )
                for nb in range(D // NB):
                    nc.vector.tensor_add(
                        out=vw[:, mk, nb * NB:(nb + 1) * NB],
                        in0=ps2[nb],
                        in1=bvo_sb[:, nb * NB:(nb + 1) * NB],
                    )
                nc.vector.tensor_scalar_mul(
                    out=vw[:, mk, :], in0=vw[:, mk, :], scalar1=recip[:, mk:mk + 1]
                )

            # ---- P4: yT[d,q] = sum_k vw[k,d] * expT[k,q]; store per chunk.
            # The very last chunk is split into two 256-wide halves so the
            # final drain is a half-size copy + store.
            for md in range(DO):
                for qb in range(NQB):
                    last = (md == DO - 1 and qb == NQB - 1)
                    widths = (384, 128) if last else (QB,)
                    q0 = qb * QB
                    for w in widths:
                        ps4 = psum.tile([P, w], F32, tag="ps",
                                        name=f"p4_{md}_{q0}")
                        for kt in range(TO):
                            nc.tensor.matmul(
                                ps4,
                                lhsT=vw[:, kt, md * P:(md + 1) * P],
                                rhs=expT[:, kt, q0:q0 + w],
                                start=(kt == 0),
                                stop=(kt == TO - 1),
                            )
                        yt = yt_pool.tile([P, w], F32, tag="yt")
                        if last and w == 128:
                            # final drain: copy on the scalar engine so the
                            # store that follows needs no cross-engine hop
                            nc.scalar.copy(out=yt, in_=ps4)
                            nc.scalar.dma_start(
                                out=y[md * P:(md + 1) * P, q0:q0 + w],
                                in_=yt,
                            )
                        else:
                            nc.vector.tensor_copy(out=yt, in_=ps4)
                            eng = nc.sync if (md * NQB + qb + q0) % 2 == 0 else nc.scalar
                            eng.dma_start(
                                out=y[md * P:(md + 1) * P, q0:q0 + w],
                                in_=yt,
                            )
                        q0 += w

    nc.finalize()
    return nc


_NC_CACHE = []


def _get_nc():
    if not _NC_CACHE:
        _NC_CACHE.append(_build_program())
    return _NC_CACHE[0]


def _numpy_fallback(query, key, value, Wq, bq, Wk, bk, Wv, bv, Wo, bo):
    f = np.float32
    q = np.einsum("bsd,hd->bsh", query, Wq).astype(f) + bq
    k = np.einsum("bsd,hd->bsh", key, Wk).astype(f) + bk
    v = np.einsum("bsd,hd->bsh", value, Wv).astype(f) + bv
    s = np.einsum("bqh,bkh->bqk", q, k) * np.float32(SCALE)
    s = s - s.max(axis=1, keepdims=True)
    e = np.exp(s)
    attn = e / e.sum(axis=1, keepdims=True)
    out = np.einsum("bqk,bkh->bqh", attn, v)
    return (np.einsum("bqh,dh->bqd", out, Wo) + bo).astype(f)


def run(query, key, value, Wq, bq, Wk, bk, Wv, bv, Wo, bo, **spmd_kwargs):
    """Run on 8 cores; returns (output [B,S,D] fp32, BassKernelResults|None)."""
    import ml_dtypes
    f = np.float32
    bf = ml_dtypes.bfloat16
    query = np.asarray(query, f)
    key = np.asarray(key, f)
    value = np.asarray(value, f)
    Wq, Wk, Wv, Wo = (np.asarray(w, f) for w in (Wq, Wk, Wv, Wo))
    bq, bk, bv, bo = (np.asarray(b_, f) for b_ in (bq, bk, bv, bo))

    if np.any(bq) or np.any(bk):
        # The G-composition absorbs the q/k projections and cannot represent
        # nonzero q/k biases; this problem's setup_inputs always has zeros.
        return _numpy_fallback(query, key, value, Wq, bq, Wk, bk, Wv, bv, Wo, bo), None

    w64 = np.float64
    gT = np.ascontiguousarray((Wk.astype(w64).T @ Wq.astype(w64))).astype(bf)  # G^T
    gvT = np.ascontiguousarray((Wv.astype(w64).T @ Wo.astype(w64).T)).astype(bf)
    bvo = (Wo.astype(w64) @ bv.astype(w64)).astype(f)

    in_maps = []
    for core in range(8):
        b, half = divmod(core, 2)
        sl = slice(half * T, (half + 1) * T)
        in_maps.append({
            "xqT": np.ascontiguousarray(query[b].T).astype(bf),      # [D, S]
            "xkT": np.ascontiguousarray(key[b, sl].T).astype(bf),    # [D, T]
            "xvT": np.ascontiguousarray(value[b, sl].T).astype(bf),  # [D, T]
            "gT": gT, "gvT": gvT, "bvo": bvo,
        })

    nc = _get_nc()
    res = run_bass_kernel_spmd(nc, in_maps, core_ids=list(range(8)), **spmd_kwargs)
    out = np.stack(
        [(res.results[2 * b]["y"] + res.results[2 * b + 1]["y"]).T + bo
         for b in range(B)]
    ).astype(f)
    return out, res


def kernel(query, key, value, Wq, bq, Wk, bk, Wv, bv, Wo, bo):
    out, _ = run(query, key, value, Wq, bq, Wk, bk, Wv, bv, Wo, bo)
    return out
